# revision 13
# baseline (speedup 1.0000x reference)
"""ALiBi bias kernel distributed across 8 TRN2 NeuronCores.

out[b,h,i,j] = scores[b,h,i,j] - slopes[h] * (pos[i] - pos[j])
             = scores[b,h,i,j] + (-slopes[h]*pos[i]) + (slopes[h]*pos[j])

Pure data-parallel: the 32 (b,h) slices are split 4 per core. Per core we
stream 64 MiB in + 64 MiB out; the row bias -slopes*pos[i] is a
per-partition scalar and the column bias +slopes*pos[j] a broadcast row,
so one DVE scalar_tensor_tensor per tile does the whole compute.
"""

import numpy as np

import concourse.bass as bass
import concourse.bacc as bacc
import concourse.mybir as mybir
import concourse.tile as tile
from concourse.bass_utils import run_bass_kernel_spmd

NC = 8                 # NeuronCores
B, H, S = 2, 16, 2048  # scores: [B, H, S, S]
G = B * H              # 32 global (b,h) slices
GP = G // NC           # 4 slices per core
P = 128                # SBUF partitions
NBLK = S // P          # 16 row-blocks per slice
BLKS = GP * NBLK       # 64 row-blocks per core
KB = 2                 # row-blocks per DMA (2 x 1 MiB = 2 MiB per transfer)
F32 = mybir.dt.float32


def build(bufs: int = 4, kb: int = KB):
    """Per-core Bass graph. Same graph on all 8 cores; data differs."""
    nc = bacc.Bacc()
    scores_ext = nc.declare_dram_parameter("scores", [BLKS * P, S], F32, isOutput=False)
    # consts = [negr (BLKS cols) | cmat (GP*S cols)] fused into one DMA so
    # compute ops carry at most 2 sync waits (scores DMA + consts DMA).
    consts_ext = nc.declare_dram_parameter("consts", [P, BLKS + GP * S], F32, isOutput=False)
    out_ext = nc.declare_dram_parameter("out", [BLKS * P, S], F32, isOutput=True)

    sc_v = scores_ext[:, :].rearrange("(n p) s -> p n s", p=P)   # [128, 64, 2048]
    out_v = out_ext[:, :].rearrange("(n p) s -> p n s", p=P)

    with tile.TileContext(nc) as tc:
        with (
            tc.tile_pool(name="const", bufs=1) as cpool,
            tc.tile_pool(name="work", bufs=bufs) as wpool,
            tc.tile_pool(name="outp", bufs=bufs) as opool,
        ):
            consts_t = cpool.tile([P, BLKS + GP * S], F32, tag="consts")
            nc.sync.dma_start(consts_t[:, :], consts_ext[:, :])
            # Absorb the consts-DMA dependency into DVE's clock here, so the
            # STT ops below carry only their scores-DMA wait (the STT struct
            # has a single sync-wait slot).
            scratch_t = cpool.tile([P, 1], F32, tag="scratch")
            nc.vector.tensor_copy(scratch_t[:, :], consts_t[:, 0:1])

            for d in range(BLKS // kb):
                t = wpool.tile([P, kb * S], F32, tag="t")
                t3d = t[:, :].rearrange("p (n s) -> p n s", s=S)
                nc.sync.dma_start(t3d, sc_v[:, d * kb:(d + 1) * kb, :])
                t2 = opool.tile([P, kb * S], F32, tag="t2")
                # Pre-touch: absorb the in-DMA wait into DVE's clock so the
                # STTs below stay within the 2-sync-wait struct limit.
                nc.vector.tensor_copy(scratch_t[:, :], t[:, 0:1])
                for b in range(kb):
                    blk = d * kb + b
                    g = blk // NBLK  # local slice index on this core
                    nc.vector.scalar_tensor_tensor(
                        t2[:, b * S:(b + 1) * S],
                        t[:, b * S:(b + 1) * S],
                        consts_t[:, blk:blk + 1],
                        consts_t[:, BLKS + g * S:BLKS + (g + 1) * S],
                        op0=mybir.AluOpType.add,
                        op1=mybir.AluOpType.add,
                    )
                t23d = t2[:, :].rearrange("p (n s) -> p n s", s=S)
                nc.sync.dma_start(out_v[:, d * kb:(d + 1) * kb, :], t23d)
    nc.compile()
    return nc


def make_in_maps(scores, slopes, positions, offset=0):
    scores = np.asarray(scores, dtype=np.float32).reshape(G, S, S)
    slopes = np.asarray(slopes, dtype=np.float32).reshape(H)
    positions = np.asarray(positions, dtype=np.float32)
    off = float(np.asarray(offset))
    pos = positions[:S] + np.float32(off)
    slopes_g = np.broadcast_to(slopes[None, :], (B, H)).reshape(G)

    in_maps = []
    for c in range(NC):
        sc = scores[c * GP:(c + 1) * GP].reshape(GP * S, S)
        consts = np.empty((P, BLKS + GP * S), np.float32)
        for li in range(GP):
            r = slopes_g[c * GP + li] * pos          # [S] = slope * pos
            consts[:, li * NBLK:(li + 1) * NBLK] = -r.reshape(NBLK, P).T
            consts[:, BLKS + li * S:BLKS + (li + 1) * S] = r[None, :]
        in_maps.append({"scores": sc, "consts": consts})
    return in_maps


def kernel(**inputs):
    in_maps = make_in_maps(
        inputs["scores"], inputs["slopes"], inputs["positions"],
        inputs.get("offset", 0),
    )
    nc = build()
    res = run_bass_kernel_spmd(nc, in_maps, core_ids=list(range(NC)))
    out = np.concatenate(
        [np.asarray(res.results[c]["out"]).reshape(GP, S, S) for c in range(NC)],
        axis=0,
    )
    return out.reshape(B, H, S, S)


# revision 19
# speedup vs baseline: 1.0803x; 1.0803x over previous
"""ALiBi bias kernel distributed across 8 TRN2 NeuronCores.

out[b,h,i,j] = scores[b,h,i,j] - slopes[h] * (pos[i] - pos[j])
             = scores[b,h,i,j] + (-slopes[h]*pos[i]) + (slopes[h]*pos[j])

Pure data-parallel: the 32 (b,h) slices are split 4 per core. Per core we
stream 64 MiB in + 64 MiB out; the row bias -slopes*pos[i] is a
per-partition scalar and the column bias +slopes*pos[j] a broadcast row,
so one DVE scalar_tensor_tensor per row-block does the whole compute.
In-DMAs ride the SP HWDGE ring and out-DMAs the ACT ring so the two
streams can't head-of-line block each other.
"""

import numpy as np

import concourse.bass as bass
import concourse.bacc as bacc
import concourse.mybir as mybir
import concourse.tile as tile
from concourse.bass_utils import run_bass_kernel_spmd

NC = 8                 # NeuronCores
B, H, S = 2, 16, 2048  # scores: [B, H, S, S]
G = B * H              # 32 global (b,h) slices
GP = G // NC           # 4 slices per core
P = 128                # SBUF partitions
NBLK = S // P          # 16 row-blocks per slice
BLKS = GP * NBLK       # 64 row-blocks per core
F32 = mybir.dt.float32


def build(kb: int = 4, bufs: int = 4, split_rings: bool = True, inplace: bool = True,
          bcast: bool = True):
    """Per-core Bass graph. Same graph on all 8 cores; data differs.

    kb: row-blocks per DMA transfer (kb MiB per dma_start)
    split_rings: out-DMAs on the ACT HWDGE ring instead of SP
    inplace: STT writes back into the input tile (halves SBUF, serializes
             out-DMA behind the whole tile's compute)
    bcast: consts input carries only [1, ...] column-bias rows; the
           [128, GP*S] broadcast tile is built on-chip via gpsimd
           partition_broadcast (saves ~4.1 MB of HBM traffic)
    """
    nc = bacc.Bacc()
    scores_ext = nc.declare_dram_parameter("scores", [BLKS * P, S], F32, isOutput=False)
    if bcast:
        negr_ext = nc.declare_dram_parameter("negr", [P, BLKS], F32, isOutput=False)
        crow_ext = nc.declare_dram_parameter("crow", [1, GP * S], F32, isOutput=False)
    else:
        consts_ext = nc.declare_dram_parameter("consts", [P, BLKS + GP * S], F32, isOutput=False)
    out_ext = nc.declare_dram_parameter("out", [BLKS * P, S], F32, isOutput=True)

    sc_v = scores_ext[:, :].rearrange("(n p) s -> p n s", p=P)   # [128, 64, 2048]
    out_v = out_ext[:, :].rearrange("(n p) s -> p n s", p=P)
    out_eng = nc.scalar if split_rings else nc.sync

    with tile.TileContext(nc) as tc:
        with (
            tc.tile_pool(name="const", bufs=1) as cpool,
            tc.tile_pool(name="work", bufs=bufs) as wpool,
            tc.tile_pool(name="outp", bufs=bufs) as opool,
        ):
            consts_t = cpool.tile([P, BLKS + GP * S], F32, tag="consts")
            if bcast:
                nc.sync.dma_start(consts_t[:, 0:BLKS], negr_ext[:, :])
                nc.sync.dma_start(consts_t[0:1, BLKS:], crow_ext[:, :])
                nc.gpsimd.partition_broadcast(
                    consts_t[:, BLKS:], consts_t[0:1, BLKS:])
            else:
                nc.sync.dma_start(consts_t[:, :], consts_ext[:, :])

            for d in range(BLKS // kb):
                t = wpool.tile([P, kb * S], F32, tag="t")
                t3d = t[:, :].rearrange("p (n s) -> p n s", s=S)
                nc.sync.dma_start(t3d, sc_v[:, d * kb:(d + 1) * kb, :])
                t2 = t if inplace else opool.tile([P, kb * S], F32, tag="t2")
                for b in range(kb):
                    blk = d * kb + b
                    g = blk // NBLK  # local slice index on this core
                    nc.vector.scalar_tensor_tensor(
                        t2[:, b * S:(b + 1) * S],
                        t[:, b * S:(b + 1) * S],
                        consts_t[:, blk:blk + 1],
                        consts_t[:, BLKS + g * S:BLKS + (g + 1) * S],
                        op0=mybir.AluOpType.add,
                        op1=mybir.AluOpType.add,
                    )
                t23d = t2[:, :].rearrange("p (n s) -> p n s", s=S)
                out_eng.dma_start(out_v[:, d * kb:(d + 1) * kb, :], t23d)
    nc.compile()
    return nc


def make_in_maps(scores, slopes, positions, offset=0, bcast=False):
    scores = np.asarray(scores, dtype=np.float32).reshape(G, S, S)
    slopes = np.asarray(slopes, dtype=np.float32).reshape(H)
    positions = np.asarray(positions, dtype=np.float32)
    off = float(np.asarray(offset))
    pos = positions[:S] + np.float32(off)
    slopes_g = np.broadcast_to(slopes[None, :], (B, H)).reshape(G)

    in_maps = []
    for c in range(NC):
        sc = scores[c * GP:(c + 1) * GP].reshape(GP * S, S)
        negr = np.empty((P, BLKS), np.float32)
        crow = np.empty((1, GP * S), np.float32)
        for li in range(GP):
            r = slopes_g[c * GP + li] * pos          # [S] = slope * pos
            negr[:, li * NBLK:(li + 1) * NBLK] = -r.reshape(NBLK, P).T
            crow[0, li * S:(li + 1) * S] = r
        if bcast:
            in_maps.append({"scores": sc, "negr": negr, "crow": crow})
        else:
            consts = np.concatenate(
                [negr, np.broadcast_to(crow, (P, GP * S))], axis=1)
            in_maps.append({"scores": sc, "consts": np.ascontiguousarray(consts)})
    return in_maps


def kernel(**inputs):
    in_maps = make_in_maps(
        inputs["scores"], inputs["slopes"], inputs["positions"],
        inputs.get("offset", 0), bcast=True,
    )
    nc = build()
    res = run_bass_kernel_spmd(nc, in_maps, core_ids=list(range(NC)))
    out = np.concatenate(
        [np.asarray(res.results[c]["out"]).reshape(GP, S, S) for c in range(NC)],
        axis=0,
    )
    return out.reshape(B, H, S, S)


# revision 20
# speedup vs baseline: 1.4267x; 1.3207x over previous
"""ALiBi bias kernel distributed across 8 TRN2 NeuronCores.

out[b,h,i,j] = scores[b,h,i,j] - slopes[h] * (pos[i] - pos[j])
             = scores[b,h,i,j] + (-slopes[h]*pos[i]) + (slopes[h]*pos[j])

Pure data-parallel: the 32 (b,h) slices are split 4 per core. Per core we
stream 64 MiB in + 64 MiB out; the row bias -slopes*pos[i] is a
per-partition scalar and the column bias +slopes*pos[j] a broadcast row,
so one DVE scalar_tensor_tensor per row-block does the whole compute.
In-DMAs ride the SP HWDGE ring and out-DMAs the ACT ring so the two
streams can't head-of-line block each other.
"""

import numpy as np

import concourse.bacc as bacc
import concourse.mybir as mybir
import concourse.tile as tile
from concourse.bass_utils import run_bass_kernel_spmd

NC = 8                 # NeuronCores
B, H, S = 2, 16, 2048  # scores: [B, H, S, S]
G = B * H              # 32 global (b,h) slices
GP = G // NC           # 4 slices per core
P = 128                # SBUF partitions
NBLK = S // P          # 16 row-blocks per slice
BLKS = GP * NBLK       # 64 row-blocks per core
F32 = mybir.dt.float32


def build(kb: int = 4, bufs: int = 4, split_rings: bool = True, inplace: bool = True,
          bcast: bool = True):
    """Per-core Bass graph. Same graph on all 8 cores; data differs.

    kb: row-blocks per DMA transfer (kb MiB per dma_start)
    split_rings: out-DMAs on the ACT HWDGE ring instead of SP
    inplace: STT writes back into the input tile (halves SBUF, serializes
             out-DMA behind the whole tile's compute)
    bcast: consts input carries only [1, ...] column-bias rows; the
           [128, GP*S] broadcast tile is built on-chip via gpsimd
           partition_broadcast (saves ~4.1 MB of HBM traffic)
    """
    nc = bacc.Bacc()
    scores_ext = nc.declare_dram_parameter("scores", [BLKS * P, S], F32, isOutput=False)
    if bcast:
        negr_ext = nc.declare_dram_parameter("negr", [P, BLKS], F32, isOutput=False)
        crow_ext = nc.declare_dram_parameter("crow", [1, GP * S], F32, isOutput=False)
    else:
        consts_ext = nc.declare_dram_parameter("consts", [P, BLKS + GP * S], F32, isOutput=False)
    out_ext = nc.declare_dram_parameter("out", [BLKS * P, S], F32, isOutput=True)

    sc_v = scores_ext[:, :].rearrange("(n p) s -> p n s", p=P)   # [128, 64, 2048]
    out_v = out_ext[:, :].rearrange("(n p) s -> p n s", p=P)
    out_eng = nc.scalar if split_rings else nc.sync

    with tile.TileContext(nc) as tc:
        with (
            tc.tile_pool(name="const", bufs=1) as cpool,
            tc.tile_pool(name="work", bufs=bufs) as wpool,
            tc.tile_pool(name="outp", bufs=bufs) as opool,
        ):
            consts_t = cpool.tile([P, BLKS + GP * S], F32, tag="consts")
            if bcast:
                nc.sync.dma_start(consts_t[:, 0:BLKS], negr_ext[:, :])
                nc.sync.dma_start(consts_t[0:1, BLKS:], crow_ext[:, :])
                nc.gpsimd.partition_broadcast(
                    consts_t[:, BLKS:], consts_t[0:1, BLKS:])
            else:
                nc.sync.dma_start(consts_t[:, :], consts_ext[:, :])

            for d in range(BLKS // kb):
                t = wpool.tile([P, kb * S], F32, tag="t")
                t3d = t[:, :].rearrange("p (n s) -> p n s", s=S)
                nc.sync.dma_start(t3d, sc_v[:, d * kb:(d + 1) * kb, :])
                t2 = t if inplace else opool.tile([P, kb * S], F32, tag="t2")
                for b in range(kb):
                    blk = d * kb + b
                    g = blk // NBLK  # local slice index on this core
                    nc.vector.scalar_tensor_tensor(
                        t2[:, b * S:(b + 1) * S],
                        t[:, b * S:(b + 1) * S],
                        consts_t[:, blk:blk + 1],
                        consts_t[:, BLKS + g * S:BLKS + (g + 1) * S],
                        op0=mybir.AluOpType.add,
                        op1=mybir.AluOpType.add,
                    )
                t23d = t2[:, :].rearrange("p (n s) -> p n s", s=S)
                out_eng.dma_start(out_v[:, d * kb:(d + 1) * kb, :], t23d)
    nc.compile()
    return nc


def make_in_maps(scores, slopes, positions, offset=0, bcast=False):
    scores = np.asarray(scores, dtype=np.float32).reshape(G, S, S)
    slopes = np.asarray(slopes, dtype=np.float32).reshape(H)
    positions = np.asarray(positions, dtype=np.float32)
    off = float(np.asarray(offset))
    pos = positions[:S] + np.float32(off)
    slopes_g = np.broadcast_to(slopes[None, :], (B, H)).reshape(G)

    in_maps = []
    for c in range(NC):
        sc = scores[c * GP:(c + 1) * GP].reshape(GP * S, S)
        negr = np.empty((P, BLKS), np.float32)
        crow = np.empty((1, GP * S), np.float32)
        for li in range(GP):
            r = slopes_g[c * GP + li] * pos          # [S] = slope * pos
            negr[:, li * NBLK:(li + 1) * NBLK] = -r.reshape(NBLK, P).T
            crow[0, li * S:(li + 1) * S] = r
        if bcast:
            in_maps.append({"scores": sc, "negr": negr, "crow": crow})
        else:
            consts = np.concatenate(
                [negr, np.broadcast_to(crow, (P, GP * S))], axis=1)
            in_maps.append({"scores": sc, "consts": np.ascontiguousarray(consts)})
    return in_maps


def kernel(**inputs):
    in_maps = make_in_maps(
        inputs["scores"], inputs["slopes"], inputs["positions"],
        inputs.get("offset", 0), bcast=True,
    )
    nc = build()
    res = run_bass_kernel_spmd(nc, in_maps, core_ids=list(range(NC)))
    out = np.concatenate(
        [np.asarray(res.results[c]["out"]).reshape(GP, S, S) for c in range(NC)],
        axis=0,
    )
    return out.reshape(B, H, S, S)
